# revision 1
# baseline (speedup 1.0000x reference)
"""GAT message-passing kernel for Trainium2 (8 NeuronCores, Bass/Tile).

Strategy (edge-parallel graph partitioning, per the sharding hint):
the model output y = elu(sum(xo[0] * xo[1:item_len], 1)) depends only on
output rows 0..item_len-1, so only edges with dst < item_len contribute.
Python partitions those edges by dst block of 128 (core k owns dst rows
[128k, 128k+128)); every core additionally processes the dst==0 edges so
xo[0] is available locally (no collectives needed).

On device, per core:
  - gather x[src] rows via indirect DMA (node features stay replicated
    in HBM; the gather is the memory-bound core of the kernel)
  - per-edge attention logits via fused multiply+row-reduce against
    broadcast W@att_src / W@att_dst vectors
  - segment softmax numerator + scatter-add via one-hot selection
    matmuls accumulated in PSUM:  acc[j,:] += S^T @ [p * x_src | p]
    (the gathered tile carries a constant ones column so one ACT copy
    with per-edge scale p produces the whole matmul rhs)
  - out = (acc_u @ W) / z + bias;  xo = elu(out)
  - y_k[j] = elu(dot(xo[0], xo[j]))  (row 0 from the dst==0 block)
Python concatenates the 8 y_k slices into the final [item_len-1] vector.
"""
import math

import numpy as np

P = 128
N_CORES = 8
NEG_SLOPE = 0.2

_CACHE = {}


def _build_program(n_nodes, in_dim, out_dim, T_main, T0):
    import concourse.bass as bass
    import concourse.bacc as bacc
    import concourse.tile as tile
    import concourse.mybir as mybir
    from concourse.masks import make_identity
    from contextlib import ExitStack

    f32 = mybir.dt.float32
    bf16 = mybir.dt.bfloat16
    i32 = mybir.dt.int32
    Alu = mybir.AluOpType
    Act = mybir.ActivationFunctionType
    IND = in_dim
    OUTD = out_dim

    nc = bacc.Bacc(
        "TRN2", target_bir_lowering=False, debug=False, num_devices=N_CORES
    )
    x_in = nc.dram_tensor("x_in", [n_nodes, IND], f32, kind="ExternalInput").ap()
    w_in = nc.dram_tensor("w_in", [IND, OUTD], f32, kind="ExternalInput").ap()
    wt_in = nc.dram_tensor("wt_in", [OUTD, IND], f32, kind="ExternalInput").ap()
    # att_src | att_dst as columns
    av_in = nc.dram_tensor("av_in", [OUTD, 2], f32, kind="ExternalInput").ap()
    bias_in = nc.dram_tensor("bias_in", [1, OUTD], f32, kind="ExternalInput").ap()
    # esrc || edst packed [P, 2T]
    eidx_in = nc.dram_tensor(
        "eidx_in", [P, 2 * T_main], i32, kind="ExternalInput"
    ).ap()
    bidx_in = nc.dram_tensor("bidx_in", [P, 2 * T0], i32, kind="ExternalInput").ap()
    # main rows | block-B rows
    rows_in = nc.dram_tensor("rows_in", [P, 2], i32, kind="ExternalInput").ap()
    # [P, 2P]: identity matrix || iota-along-free (both f32 constants)
    consts_in = nc.dram_tensor(
        "consts_in", [P, 2 * P], f32, kind="ExternalInput"
    ).ap()
    y_out = nc.dram_tensor("y_out", [1, P], f32, kind="ExternalOutput").ap()

    with tile.TileContext(nc) as tc, ExitStack() as ctx:
        const = ctx.enter_context(tc.tile_pool(name="const", bufs=1))
        idxp = ctx.enter_context(tc.tile_pool(name="idx", bufs=1))
        xgp = ctx.enter_context(tc.tile_pool(name="xg", bufs=T_main + T0 + 2))
        sp = ctx.enter_context(tc.tile_pool(name="sp", bufs=4))
        scrp = ctx.enter_context(tc.tile_pool(name="scr", bufs=3))
        rhsp = ctx.enter_context(tc.tile_pool(name="rhs", bufs=4))
        smallp = ctx.enter_context(tc.tile_pool(name="small", bufs=4))
        accsbp = ctx.enter_context(tc.tile_pool(name="accsb", bufs=2))
        xop = ctx.enter_context(tc.tile_pool(name="xop", bufs=2))
        # PSUM banks: acc 2 + tp 3 + outp 2 = 7
        accp = ctx.enter_context(tc.tile_pool(name="acc", bufs=2, space="PSUM"))
        tpp = ctx.enter_context(tc.tile_pool(name="tp", bufs=3, space="PSUM"))
        outpp = ctx.enter_context(tc.tile_pool(name="outp", bufs=2, space="PSUM"))

        # ---- index DMAs first: the gather stream depends only on these ----
        rows_t = idxp.tile([P, 2], i32, tag="rows")
        nc.sync.dma_start(rows_t[:], rows_in[:])
        bidx_t = idxp.tile([P, 2 * T0], i32, tag="bidx")
        nc.sync.dma_start(bidx_t[:], bidx_in[:])
        eidx_t = idxp.tile([P, 2 * T_main], i32, tag="eidx")
        nc.sync.dma_start(eidx_t[:], eidx_in[:])

        # ---- constants (identity/iota DMA'd: keeps gpsimd free for gathers) ----
        cc = const.tile([P, 2 * P], f32, tag="cc")
        nc.sync.dma_start(cc[:], consts_in[:])
        ident = cc[:, 0:P]
        iota_f = cc[:, P : 2 * P]
        ones_row = const.tile([1, P], f32, tag="ones_row")
        nc.vector.memset(ones_row[:], 1.0)

        W0 = const.tile([P, OUTD], f32, tag="W0")
        nc.sync.dma_start(W0[:], w_in[0:P, :])
        W1 = const.tile([P, OUTD], f32, tag="W1")
        nc.sync.dma_start(W1[:], w_in[P : 2 * P, :])
        av_col = const.tile([OUTD, 2], f32, tag="av_col")
        nc.sync.dma_start(av_col[:], av_in[:])
        bias_row = const.tile([1, OUTD], f32, tag="bias_row")
        nc.sync.dma_start(bias_row[:], bias_in[:])

        # W^T is an input layout: w_att row = att_col^T @ W^T in one matmul
        WTf = const.tile([P, IND], f32, tag="WTf")
        nc.sync.dma_start(WTf[:], wt_in[:])

        def proj_row(att_col, name):
            # [1, IND] row of W @ att  =  att^T @ W^T
            rowp = tpp.tile([1, IND], f32, tag="tp")
            nc.tensor.matmul(
                rowp[:], lhsT=att_col, rhs=WTf[:], start=True, stop=True
            )
            row = const.tile([1, IND], f32, tag=f"{name}_row")
            nc.vector.tensor_copy(row[:], rowp[:])
            return row

        ws_row = proj_row(av_col[:, 0:1], "ws")
        wd_row = proj_row(av_col[:, 1:2], "wd")

        def bcast_rows(row_sb, width, name):
            # [1, width] -> [P, width] via rank-1 matmul
            bp = tpp.tile([P, IND], f32, tag="tp")
            nc.tensor.matmul(
                bp[:, :width], lhsT=ones_row[:], rhs=row_sb[:, :width],
                start=True, stop=True,
            )
            bs = const.tile([P, width], f32, tag=f"{name}_b")
            nc.vector.tensor_copy(bs[:], bp[:, :width])
            return bs

        wsb = bcast_rows(ws_row, IND, "wsb")
        wdb = bcast_rows(wd_row, IND, "wdb")
        bias_b = bcast_rows(bias_row, OUTD, "bias")

        def emit_block(rows_ap, idx_t, T, tag, self_loop_last=False):
            xb = xgp.tile([P, IND], f32, tag="xb")
            nc.gpsimd.indirect_dma_start(
                out=xb[:],
                out_offset=None,
                in_=x_in[:],
                in_offset=bass.IndirectOffsetOnAxis(ap=rows_ap, axis=0),
            )
            adb = smallp.tile([P, 1], f32, tag="adb")
            scr_b = scrp.tile([P, IND], bf16, tag="scr")
            nc.vector.scalar_tensor_tensor(
                out=scr_b[:], in0=xb[:], scalar=0.0, in1=wdb[:],
                op0=Alu.bypass, op1=Alu.mult, accum_out=adb[:],
            )
            # broadcast a_d along partitions: adb_b[e, j] = a_d[j]
            adrp = tpp.tile([P, P], f32, tag="tp")
            nc.tensor.transpose(adrp[:1, :], adb[:], ident)
            adr = const.tile([1, P], f32, tag=f"adr{tag}")
            nc.vector.tensor_copy(adr[:], adrp[:1, :P])
            adbp = tpp.tile([P, P], f32, tag="tp")
            nc.tensor.matmul(
                adbp[:], lhsT=ones_row[:], rhs=adr[:], start=True, stop=True
            )
            adb_b = const.tile([P, P], f32, tag=f"adb_b{tag}")
            nc.vector.tensor_copy(adb_b[:], adbp[:])
            dstf_all = idxp.tile([P, T], f32, tag=f"dstf{tag}")
            nc.vector.tensor_copy(dstf_all[:], idx_t[:, T : 2 * T])

            acc = accp.tile([P, IND + 1], f32, tag="acc")
            for t in range(T):
                if self_loop_last and t == T - 1:
                    xg = xb
                else:
                    xg = xgp.tile([P, IND], f32, tag="xg")
                    nc.gpsimd.indirect_dma_start(
                        out=xg[:],
                        out_offset=None,
                        in_=x_in[:],
                        in_offset=bass.IndirectOffsetOnAxis(
                            ap=idx_t[:, t : t + 1], axis=0
                        ),
                    )
                dcol = dstf_all[:, t : t + 1]
                # Sw[e,j] = (j == dst_e) * a_d[j]; its row-sum is a_d[dst_e].
                # The common column factor a_d[j] cancels in out = (u@W)/z,
                # so Sw serves directly as the scatter matmul lhsT.
                ad_e = smallp.tile([P, 1], f32, tag="ad_e")
                Sw = sp.tile([P, P], f32, tag="Sw")
                nc.vector.scalar_tensor_tensor(
                    out=Sw[:], in0=iota_f, scalar=dcol, in1=adb_b[:],
                    op0=Alu.is_equal, op1=Alu.mult, accum_out=ad_e[:],
                )
                a_s = smallp.tile([P, 1], f32, tag="a_s")
                scr = scrp.tile([P, IND], bf16, tag="scr")
                nc.vector.scalar_tensor_tensor(
                    out=scr[:], in0=xg[:], scalar=0.0, in1=wsb[:],
                    op0=Alu.bypass, op1=Alu.mult, accum_out=a_s[:],
                )
                v = smallp.tile([P, 1], f32, tag="v")
                nc.vector.tensor_tensor(out=v[:], in0=a_s[:], in1=ad_e[:], op=Alu.add)
                e = smallp.tile([P, 1], f32, tag="e")
                nc.vector.scalar_tensor_tensor(
                    out=e[:], in0=v[:], scalar=NEG_SLOPE, in1=v[:],
                    op0=Alu.mult, op1=Alu.max,
                )
                rhs = rhsp.tile([P, IND + 1], f32, tag="rhs")
                # Exp writes p straight into the rhs z-column; the Copy then
                # scales the gathered rows by that same column
                nc.scalar.activation(rhs[:, IND : IND + 1], e[:], Act.Exp)
                nc.scalar.activation(
                    rhs[:, 0:IND], xg[:], Act.Copy, scale=rhs[:, IND : IND + 1]
                )
                nc.tensor.matmul(
                    acc[:], lhsT=Sw[:], rhs=rhs[:],
                    start=(t == 0), stop=(t == T - 1), skip_group_check=True,
                )

            acc_sb = accsbp.tile([P, IND + 1], f32, tag="acc_sb")
            nc.vector.tensor_copy(acc_sb[:], acc[:])
            outp = outpp.tile([P, OUTD], f32, tag="outp")
            for ci in range(2):
                utp = tpp.tile([P, P], f32, tag="tp")
                nc.tensor.transpose(
                    utp[:], acc_sb[:, ci * P : (ci + 1) * P], ident
                )
                uT = sp.tile([P, P], f32, tag="uT")
                nc.vector.tensor_copy(uT[:], utp[:])
                nc.tensor.matmul(
                    outp[:], lhsT=uT[:], rhs=(W0 if ci == 0 else W1)[:],
                    start=(ci == 0), stop=(ci == 1), skip_group_check=True,
                )
            zeps = smallp.tile([P, 1], f32, tag="zeps")
            nc.vector.tensor_scalar_add(zeps[:], acc_sb[:, IND : IND + 1], 1e-30)
            rz = smallp.tile([P, 1], f32, tag="rz")
            nc.vector.reciprocal(rz[:], zeps[:])
            outn = xop.tile([P, OUTD], f32, tag="outn")
            nc.vector.scalar_tensor_tensor(
                out=outn[:], in0=outp[:], scalar=rz[:], in1=bias_b[:],
                op0=Alu.mult, op1=Alu.add,
            )
            # elu(x) = max(exp(min(x, 0)) - 1, x)
            tneg = xop.tile([P, OUTD], f32, tag="tneg")
            nc.vector.tensor_scalar_min(tneg[:], outn[:], 0.0)
            texp = xop.tile([P, OUTD], f32, tag="texp")
            nc.scalar.activation(texp[:], tneg[:], Act.Exp)
            xo = xop.tile([P, OUTD], f32, tag="xo")
            nc.vector.scalar_tensor_tensor(
                out=xo[:], in0=texp[:], scalar=-1.0, in1=outn[:],
                op0=Alu.add, op1=Alu.max,
            )
            return xo

        xo_b = emit_block(rows_t[:, 1:2], bidx_t, T0, "b")
        xo_m = emit_block(rows_t[:, 0:1], eidx_t, T_main, "m",
                          self_loop_last=True)

        xo0b = tpp.tile([P, P], f32, tag="tp")
        nc.tensor.matmul(
            xo0b[:, :OUTD], lhsT=ones_row[:], rhs=xo_b[0:1, :], start=True, stop=True
        )
        xo0s = sp.tile([P, OUTD], f32, tag="xo0s")
        nc.vector.tensor_copy(xo0s[:], xo0b[:, :OUTD])
        dscr = sp.tile([P, OUTD], bf16, tag="dscr")
        d_sb = smallp.tile([P, 1], f32, tag="d")
        nc.vector.scalar_tensor_tensor(
            out=dscr[:], in0=xo_m[:], scalar=0.0, in1=xo0s[:],
            op0=Alu.bypass, op1=Alu.mult, accum_out=d_sb[:],
        )
        yneg = smallp.tile([P, 1], f32, tag="yneg")
        nc.vector.tensor_scalar_min(yneg[:], d_sb[:], 0.0)
        yexp = smallp.tile([P, 1], f32, tag="yexp")
        nc.scalar.activation(yexp[:], yneg[:], Act.Exp)
        y_sb = smallp.tile([P, 1], f32, tag="y_sb")
        nc.vector.scalar_tensor_tensor(
            out=y_sb[:], in0=yexp[:], scalar=-1.0, in1=d_sb[:],
            op0=Alu.add, op1=Alu.max,
        )
        # write y as a contiguous [1, P] row: a [P, 1] column DMA (4B per
        # partition) has a multi-microsecond HBM completion delay that the
        # kernel-tail barrier then waits out
        yrp = tpp.tile([P, P], f32, tag="tp")
        nc.tensor.transpose(yrp[:1, :], y_sb[:], ident)
        y_row = sp.tile([1, P], f32, tag="y_row")
        nc.vector.tensor_copy(y_row[:], yrp[:1, :P])
        nc.sync.dma_start(y_out[:], y_row[:])

    nc.compile()
    return nc


def _get_program(n_nodes, in_dim, out_dim, T_main, T0):
    key = (n_nodes, in_dim, out_dim, T_main, T0)
    if key not in _CACHE:
        _CACHE[key] = _build_program(n_nodes, in_dim, out_dim, T_main, T0)
    return _CACHE[key]


def _pack_edges(src, dst_local, T):
    npad = T * P - len(src)
    s = np.concatenate([src, np.zeros(npad, np.int32)])
    d = np.concatenate([dst_local, np.full(npad, P, np.int32)])
    return np.concatenate(
        [
            np.ascontiguousarray(s.reshape(T, P).T),
            np.ascontiguousarray(d.reshape(T, P).T),
        ],
        axis=1,
    )


def prepare(x, edge_index, W, att_src, att_dst, bias, item_len):
    """Python-side graph partitioning; returns (nc, in_maps, item_len)."""
    item_len = int(np.asarray(item_len))
    x = np.ascontiguousarray(np.asarray(x, np.float32))
    W = np.ascontiguousarray(np.asarray(W, np.float32))
    att_src = np.asarray(att_src, np.float32)
    att_dst = np.asarray(att_dst, np.float32)
    bias = np.asarray(bias, np.float32)
    n_nodes, in_dim = x.shape
    out_dim = W.shape[1]
    assert item_len <= N_CORES * P, "kernel supports item_len <= 1024"

    src = np.asarray(edge_index[0])
    dst = np.asarray(edge_index[1])
    keep = dst < item_len
    src_f = src[keep].astype(np.int32)
    dst_f = dst[keep].astype(np.int32)
    loops = np.arange(item_len, dtype=np.int32)
    src_all = np.concatenate([src_f, loops])
    dst_all = np.concatenate([dst_f, loops])

    blk = dst_f // P  # graph edges only; self-loop tile appended per core
    order = np.argsort(blk, kind="stable")
    src_f = src_f[order]
    dst_f = dst_f[order]
    blk = blk[order]
    bounds = np.searchsorted(blk, np.arange(N_CORES + 1))
    counts = np.diff(bounds)
    # +1: last tile holds exactly the 128 self-loop edges (reuses block rows)
    T_main = max(1, int(math.ceil(counts.max() / P))) + 1

    sel0 = dst_all == 0
    b0_src = src_all[sel0]
    b0_dst = dst_all[sel0]
    T0 = max(1, int(math.ceil(len(b0_src) / P)))
    bidx = _pack_edges(b0_src, b0_dst, T0)
    brows = np.arange(P, dtype=np.int32)

    nc = _get_program(n_nodes, in_dim, out_dim, T_main, T0)

    av = np.ascontiguousarray(np.stack([att_src, att_dst], axis=1))  # [OUTD, 2]
    consts = np.concatenate(
        [np.eye(P, dtype=np.float32),
         np.tile(np.arange(P, dtype=np.float32)[None, :], (P, 1))], axis=1
    )
    in_maps = []
    for k in range(N_CORES):
        lo, hi = bounds[k], bounds[k + 1]
        mrows_flat = np.minimum(
            np.arange(k * P, (k + 1) * P, dtype=np.int32), n_nodes - 1
        )
        es = src_f[lo:hi]
        ed = dst_f[lo:hi] - k * P
        npad = (T_main - 1) * P - len(es)
        es = np.concatenate([es, np.zeros(npad, np.int32), mrows_flat])
        loop_dst = np.arange(P, dtype=np.int32)
        if (k + 1) * P > item_len:  # rows beyond item_len get no self-loop
            loop_dst = np.where(
                np.arange(k * P, (k + 1) * P) < item_len, loop_dst, P
            ).astype(np.int32)
        ed = np.concatenate([ed, np.full(npad, P, np.int32), loop_dst])
        eidx = _pack_edges(es, ed, T_main)
        in_maps.append(
            {
                "x_in": x,
                "w_in": W,
                "wt_in": np.ascontiguousarray(W.T),
                "av_in": av,
                "bias_in": np.ascontiguousarray(bias.reshape(1, out_dim)),
                "eidx_in": eidx,
                "bidx_in": bidx,
                "rows_in": np.ascontiguousarray(
                    np.stack([mrows_flat, brows], axis=1)
                ),
                "consts_in": consts,
            }
        )
    return nc, in_maps, item_len


def assemble(results, item_len):
    y_all = np.concatenate([results[k]["y_out"].ravel() for k in range(N_CORES)])
    return y_all[1:item_len].astype(np.float32)


def kernel(x, edge_index, W, att_src, att_dst, bias, item_len):
    from concourse import bass_utils

    nc, in_maps, item_len = prepare(
        x, edge_index, W, att_src, att_dst, bias, item_len
    )
    res = bass_utils.run_bass_kernel_spmd(nc, in_maps, core_ids=list(range(N_CORES)))
    return assemble(res.results, item_len)



# revision 19
# speedup vs baseline: 1.6838x; 1.6838x over previous
"""GAT message-passing kernel for Trainium2 (8 NeuronCores, Bass/Tile).

Strategy v4 (edge-parallel, host-packed gather, gpsimd one-hot scatter):
y = elu(sum(xo[0] * xo[1:item_len], 1)) depends only on output rows
0..item_len-1, so only edges with dst < item_len contribute (~33.7k of
3.2M).  The host filters those edges, partitions them by dst block of
128 (core k owns dst rows [128k, 128k+128)), gathers x[src] rows and
packs them bf16 as [128 edges/tile | 256 feats + ones column] tiles.
Edges are arranged so every 16-partition group of a tile shares one dst
(~20% padding) -- this lets ONE gpsimd indirect_copy fetch a_d[dst] for
every edge slot (its indices are shared per 16-partition group).
The device kernel streams contiguous DMA (no indirect gathers):

  setup:  a_d[j] for the core's dst rows (from the x0 tile), broadcast
          along free -> adb_b; ONE gpsimd indirect_copy gathers
          ad[e,t] = a_d[dst_e] for all tiles.
  per chunk of tiles (pipelined across engines):
    DVE:    a_s[:,t] = rowsum(xg_t * (W@att_src))        (stt accum)
    DVE:    v = a_s + ad; e = lrelu(v)                   (batched)
    Act:    p = exp(e)  -> bf16                          (batched)
    GpSimd: Sp[e, 128*i + dst] = p[e,i]  (local_scatter, one op/chunk;
            pad edges use index -1 and are dropped)
    PE:     acc += Sp_i^T @ [xg_i | 1]   (PSUM accumulate, bf16)
  epilogue: u = acc[:, :256], z = acc[:, 256];
  out = (u@W)/z + bias; xo = elu(out); y_k[j] = elu(dot(xo0, xo_j)).
  Every core also processes the dst==0 edges (block B, no one-hot:
  uB = p^T @ [xg|1]) so xo0 is available locally; Python concatenates
  the 8 y_k slices.
"""
import math

import numpy as np

P = 128
N_CORES = 8
NEG_SLOPE = 0.2
IND = 256
OUTD = 128

_CACHE = {}


def _chunk_sizes(T):
    # even-sized chunks; small first (fast pipeline start), small last
    sizes = [2]
    rem = T - 2
    while rem > 10:
        sizes.append(8)
        rem -= 8
    if rem > 2:
        sizes.append(rem - 2)
        sizes.append(2)
    elif rem > 0:
        sizes.append(rem)
    return sizes


def _build_program(T, TB, chunks):
    import concourse.bass as bass
    import concourse.bacc as bacc
    import concourse.tile as tile
    import concourse.mybir as mybir
    from contextlib import ExitStack

    f32 = mybir.dt.float32
    bf16 = mybir.dt.bfloat16
    i16 = mybir.dt.int16
    u16 = mybir.dt.uint16
    Alu = mybir.AluOpType
    Act = mybir.ActivationFunctionType
    W257 = IND + 1

    nc = bacc.Bacc(
        "TRN2", target_bir_lowering=False, debug=False, num_devices=N_CORES
    )
    C = (T + 15) // 16
    xg_in = nc.dram_tensor("xg_in", [P, T * W257], bf16, kind="ExternalInput").ap()
    xb_in = nc.dram_tensor("xb_in", [P, TB * W257], bf16, kind="ExternalInput").ap()
    x0_in = nc.dram_tensor("x0_in", [P, IND], bf16, kind="ExternalInput").ap()
    # scatter idx (chunk-local offsets, -1 pads) | gather idx (group-wrapped)
    dsti_in = nc.dram_tensor("dsti_in", [P, T], i16, kind="ExternalInput").ap()
    dstg_in = nc.dram_tensor("dstg_in", [P, C], u16, kind="ExternalInput").ap()
    maskb_in = nc.dram_tensor("maskb_in", [P, TB], f32, kind="ExternalInput").ap()
    # W halves side by side: [128, 2*128] bf16 (W[0:128,:] | W[128:256,:])
    w_in = nc.dram_tensor("w_in", [P, 2 * OUTD], bf16, kind="ExternalInput").ap()
    wt_in = nc.dram_tensor("wt_in", [OUTD, IND], f32, kind="ExternalInput").ap()
    av_in = nc.dram_tensor("av_in", [OUTD, 2], f32, kind="ExternalInput").ap()
    bias_in = nc.dram_tensor("bias_in", [1, OUTD], f32, kind="ExternalInput").ap()
    ident_in = nc.dram_tensor("ident_in", [P, P], f32, kind="ExternalInput").ap()
    y_out = nc.dram_tensor("y_out", [1, P], f32, kind="ExternalOutput").ap()

    with tile.TileContext(nc) as tc, ExitStack() as ctx:
        const = ctx.enter_context(tc.tile_pool(name="const", bufs=1))
        xgp = ctx.enter_context(tc.tile_pool(name="xg", bufs=1))
        colp = ctx.enter_context(tc.tile_pool(name="col", bufs=1))
        spp = ctx.enter_context(tc.tile_pool(name="sp", bufs=3))
        scrp = ctx.enter_context(tc.tile_pool(name="scr", bufs=2))
        smallp = ctx.enter_context(tc.tile_pool(name="small", bufs=1))
        epip = ctx.enter_context(tc.tile_pool(name="epi", bufs=1))
        # PSUM: acc 1 + uB 1 + tp 3 + out 2(tags x1) = 7 banks
        accp = ctx.enter_context(tc.tile_pool(name="acc", bufs=1, space="PSUM"))
        ubp = ctx.enter_context(tc.tile_pool(name="ub", bufs=1, space="PSUM"))
        tpp = ctx.enter_context(tc.tile_pool(name="tp", bufs=3, space="PSUM"))
        outpp = ctx.enter_context(tc.tile_pool(name="outp", bufs=1, space="PSUM"))

        # ---- DMAs ----
        ident = const.tile([P, P], f32, tag="ident")
        nc.sync.dma_start(ident[:], ident_in[:])
        WTf = const.tile([P, IND], f32, tag="WTf")
        nc.sync.dma_start(WTf[:], wt_in[:])
        av_col = const.tile([OUTD, 2], f32, tag="av_col")
        nc.sync.dma_start(av_col[:], av_in[:])
        bias_row = const.tile([1, OUTD], f32, tag="bias_row")
        nc.sync.dma_start(bias_row[:], bias_in[:])
        dsti_t = colp.tile([P, T], i16, tag="dsti")
        nc.sync.dma_start(dsti_t[:], dsti_in[:])
        dstg_t = colp.tile([P, C], u16, tag="dstg")
        nc.sync.dma_start(dstg_t[:], dstg_in[:])
        x0_t = xgp.tile([P, IND], bf16, tag="x0")
        nc.sync.dma_start(x0_t[:], x0_in[:])
        maskb_t = colp.tile([P, TB], f32, tag="maskb")
        nc.sync.dma_start(maskb_t[:], maskb_in[:])
        Wb = const.tile([P, 2 * OUTD], bf16, tag="Wb")
        nc.sync.dma_start(Wb[:], w_in[:])
        xb_t = xgp.tile([P, TB * W257], bf16, tag="xb")
        nc.sync.dma_start(xb_t[:], xb_in[:])
        xg_tiles = []
        xg_chunk = []
        off = 0
        for ci, n in enumerate(chunks):
            xc = xgp.tile([P, n * W257], bf16, tag=f"xg{ci}")
            nc.sync.dma_start(xc[:], xg_in[:, off * W257 : (off + n) * W257])
            xg_chunk.append(xc)
            for i in range(n):
                xg_tiles.append((xc, i))
            off += n

        ones_row = const.tile([1, P], f32, tag="ones_row")
        nc.vector.memset(ones_row[:], 1.0)

        # ---- setup: ws/wd rows -> broadcast [P, IND] bf16 ----
        def proj_row(att_col, name):
            rowp = tpp.tile([1, IND], f32, tag="tp")
            nc.tensor.matmul(rowp[:], lhsT=att_col, rhs=WTf[:], start=True, stop=True)
            row = const.tile([1, IND], f32, tag=f"{name}_row")
            nc.vector.tensor_copy(row[:], rowp[:])
            return row

        ws_row = proj_row(av_col[:, 0:1], "ws")
        wd_row = proj_row(av_col[:, 1:2], "wd")

        def bcast_rows(row_sb, width, name, dtype):
            bp = tpp.tile([P, IND], f32, tag="tp")
            nc.tensor.matmul(
                bp[:, :width], lhsT=ones_row[:], rhs=row_sb[:, :width],
                start=True, stop=True,
            )
            bs = const.tile([P, width], dtype, tag=f"{name}_b")
            nc.vector.tensor_copy(bs[:], bp[:, :width])
            return bs

        wsb = bcast_rows(ws_row, IND, "wsb", bf16)
        wdb = bcast_rows(wd_row, IND, "wdb", bf16)
        bias_b = bcast_rows(bias_row, OUTD, "bias", f32)

        # ---- a_d of this core's dst rows (from the x0 tile) ----
        adM = smallp.tile([P, 1], f32, tag="adM")
        scr0 = scrp.tile([P, IND], bf16, tag="scr")
        nc.vector.scalar_tensor_tensor(
            out=scr0[:], in0=x0_t[:], scalar=0.0, in1=wdb[:],
            op0=Alu.bypass, op1=Alu.mult, accum_out=adM[:],
        )
        adrp = tpp.tile([P, P], f32, tag="tp")
        nc.tensor.transpose(adrp[:1, :], adM[:], ident[:])
        adr = const.tile([1, P], f32, tag="adr")
        nc.vector.tensor_copy(adr[:], adrp[:1, :P])
        adbp = tpp.tile([P, P], f32, tag="tp")
        nc.tensor.matmul(adbp[:], lhsT=ones_row[:], rhs=adr[:], start=True, stop=True)
        adb_b = const.tile([P, P], f32, tag="adb_b")
        nc.vector.tensor_copy(adb_b[:], adbp[:])
        # gather ad[e, t] = a_d[dst(e, t)] for all tiles in one gpsimd op
        ad_all = colp.tile([P, T], f32, tag="ad_all")
        nc.gpsimd.indirect_copy(
            ad_all[:], adb_b[:], dstg_t[:], i_know_ap_gather_is_preferred=True
        )

        # ---- block B: dst==0 edges, no one-hot needed ----
        uB = ubp.tile([1, W257], f32, tag="uB")
        a_sB = smallp.tile([P, TB], f32, tag="a_sB")
        adBc = smallp.tile([P, TB], f32, tag="adBc")
        for t in range(TB):
            xbt = xb_t[:, t * W257 : t * W257 + IND]
            scrb = scrp.tile([P, IND], bf16, tag="scr")
            nc.vector.scalar_tensor_tensor(
                out=scrb[:], in0=xbt, scalar=0.0, in1=wsb[:],
                op0=Alu.bypass, op1=Alu.mult, accum_out=a_sB[:, t : t + 1],
            )
            scrb2 = scrp.tile([P, IND], bf16, tag="scr")
            nc.vector.scalar_tensor_tensor(
                out=scrb2[:], in0=xbt, scalar=0.0, in1=wdb[:],
                op0=Alu.bypass, op1=Alu.mult, accum_out=adBc[:, t : t + 1],
            )
        # ad0 = adBc[0,0] (self-loop of node 0 in slot 0) broadcast to [P,1]
        ad0p = tpp.tile([P, P], f32, tag="tp")
        nc.tensor.matmul(
            ad0p[:, 0:1], lhsT=ones_row[:], rhs=adBc[0:1, 0:1], start=True, stop=True
        )
        ad0 = smallp.tile([P, 1], f32, tag="ad0")
        nc.vector.tensor_copy(ad0[:], ad0p[:, 0:1])
        for t in range(TB):
            vB = smallp.tile([P, 1], f32, tag="vB")
            nc.vector.tensor_tensor(
                out=vB[:], in0=a_sB[:, t : t + 1], in1=ad0[:], op=Alu.add
            )
            eB = smallp.tile([P, 1], f32, tag="eB")
            nc.vector.scalar_tensor_tensor(
                out=eB[:], in0=vB[:], scalar=NEG_SLOPE, in1=vB[:],
                op0=Alu.mult, op1=Alu.max,
            )
            pB = smallp.tile([P, 1], f32, tag="pB")
            nc.scalar.activation(pB[:], eB[:], Act.Exp)
            pBm = smallp.tile([P, 1], bf16, tag="pBm")
            nc.vector.scalar_tensor_tensor(
                out=pBm[:], in0=pB[:], scalar=0.0, in1=maskb_t[:, t : t + 1],
                op0=Alu.bypass, op1=Alu.mult,
            )
            nc.tensor.matmul(
                uB[:], lhsT=pBm[:], rhs=xb_t[:, t * W257 : (t + 1) * W257],
                start=(t == 0), stop=(t == TB - 1), skip_group_check=True,
            )

        # ---- main pipeline ----
        acc = accp.tile([P, W257], f32, tag="acc")
        bounds = []
        s = 0
        for n in chunks:
            bounds.append((s, s + n))
            s += n
        for ci, (lo, hi) in enumerate(bounds):
            w = hi - lo
            as_t = smallp.tile([P, w], f32, tag=f"as{ci}")
            for t in range(lo, hi):
                xc, i = xg_tiles[t]
                scr = scrp.tile([P, IND], bf16, tag="scr")
                nc.vector.scalar_tensor_tensor(
                    out=scr[:], in0=xc[:, i * W257 : i * W257 + IND], scalar=0.0,
                    in1=wsb[:], op0=Alu.bypass, op1=Alu.mult,
                    accum_out=as_t[:, t - lo : t - lo + 1],
                )
            v_t = smallp.tile([P, w], f32, tag=f"v{ci}")
            nc.vector.tensor_tensor(
                out=v_t[:], in0=as_t[:], in1=ad_all[:, lo:hi], op=Alu.add
            )
            e_t = smallp.tile([P, w], f32, tag=f"e{ci}")
            nc.vector.scalar_tensor_tensor(
                out=e_t[:], in0=v_t[:], scalar=NEG_SLOPE, in1=v_t[:],
                op0=Alu.mult, op1=Alu.max,
            )
            p_t = smallp.tile([P, w], bf16, tag=f"p{ci}")
            nc.scalar.activation(p_t[:], e_t[:], Act.Exp)
            Sp = spp.tile([P, w * P], bf16, tag="Sp")
            nc.gpsimd.local_scatter(
                Sp[:], p_t[:], dsti_t[:, lo:hi],
                channels=P, num_elems=w * P, num_idxs=w,
            )
            for t in range(lo, hi):
                xc, i = xg_tiles[t]
                nc.tensor.matmul(
                    acc[:], lhsT=Sp[:, (t - lo) * P : (t - lo + 1) * P],
                    rhs=xc[:, i * W257 : (i + 1) * W257],
                    start=(t == 0), stop=(t == T - 1), skip_group_check=True,
                )

        # ---- main epilogue: out = (u@W)/z + bias; xo = elu(out) ----
        acc_sb = epip.tile([P, W257], f32, tag="acc_sb")
        nc.scalar.activation(acc_sb[:], acc[:], Act.Copy)
        outp = outpp.tile([P, OUTD], f32, tag="outp")
        for ci in range(2):
            utp = tpp.tile([P, P], f32, tag="tp")
            nc.tensor.transpose(utp[:], acc_sb[:, ci * P : (ci + 1) * P], ident[:])
            uT = epip.tile([P, P], bf16, tag=f"uT{ci}")
            nc.vector.tensor_copy(uT[:], utp[:])
            nc.tensor.matmul(
                outp[:], lhsT=uT[:], rhs=Wb[:, ci * OUTD : (ci + 1) * OUTD],
                start=(ci == 0), stop=(ci == 1), skip_group_check=True,
            )
        zeps = smallp.tile([P, 1], f32, tag="zeps")
        nc.vector.tensor_scalar_add(zeps[:], acc_sb[:, IND : IND + 1], 1e-30)
        rz = smallp.tile([P, 1], f32, tag="rz")
        nc.vector.reciprocal(rz[:], zeps[:])
        outn = epip.tile([P, OUTD], f32, tag="outn")
        nc.vector.scalar_tensor_tensor(
            out=outn[:], in0=outp[:], scalar=rz[:], in1=bias_b[:],
            op0=Alu.mult, op1=Alu.add,
        )
        # elu(x) = max(exp(min(x, 0)) - 1, x)
        tneg = epip.tile([P, OUTD], f32, tag="tneg")
        nc.vector.tensor_scalar_min(tneg[:], outn[:], 0.0)
        texp = epip.tile([P, OUTD], f32, tag="texp")
        nc.scalar.activation(texp[:], tneg[:], Act.Exp)
        xo = epip.tile([P, OUTD], f32, tag="xo")
        nc.vector.scalar_tensor_tensor(
            out=xo[:], in0=texp[:], scalar=-1.0, in1=outn[:],
            op0=Alu.add, op1=Alu.max,
        )

        # ---- block B epilogue: xo0 row ----
        uB_sb = epip.tile([1, W257], f32, tag="uB_sb")
        nc.vector.tensor_copy(uB_sb[:], uB[:])
        outB = outpp.tile([1, OUTD], f32, tag="outB")
        for ci in range(2):
            ubtp = tpp.tile([P, P], f32, tag="tp")
            nc.tensor.transpose(
                ubtp[:, :1], uB_sb[:1, ci * P : (ci + 1) * P], ident[0:1, 0:1]
            )
            uBT = smallp.tile([P, 1], bf16, tag=f"uBT{ci}")
            nc.vector.tensor_copy(uBT[:], ubtp[:, :1])
            nc.tensor.matmul(
                outB[:], lhsT=uBT[:], rhs=Wb[:, ci * OUTD : (ci + 1) * OUTD],
                start=(ci == 0), stop=(ci == 1), skip_group_check=True,
            )
        zBe = smallp.tile([1, 1], f32, tag="zBe")
        nc.vector.tensor_scalar_add(zBe[:], uB_sb[:1, IND : IND + 1], 1e-30)
        rzB = smallp.tile([1, 1], f32, tag="rzB")
        nc.vector.reciprocal(rzB[:], zBe[:])
        outnB = epip.tile([1, OUTD], f32, tag="outnB")
        nc.vector.scalar_tensor_tensor(
            out=outnB[:], in0=outB[:], scalar=rzB[:], in1=bias_row[:],
            op0=Alu.mult, op1=Alu.add,
        )
        tnegB = smallp.tile([1, OUTD], f32, tag="tnegB")
        nc.vector.tensor_scalar_min(tnegB[:], outnB[:], 0.0)
        texpB = smallp.tile([1, OUTD], f32, tag="texpB")
        nc.scalar.activation(texpB[:], tnegB[:], Act.Exp)
        xoB = epip.tile([1, OUTD], f32, tag="xoB")
        nc.vector.scalar_tensor_tensor(
            out=xoB[:], in0=texpB[:], scalar=-1.0, in1=outnB[:],
            op0=Alu.add, op1=Alu.max,
        )
        xo0p = tpp.tile([P, P], f32, tag="tp")
        nc.tensor.matmul(
            xo0p[:, :OUTD], lhsT=ones_row[:], rhs=xoB[:], start=True, stop=True
        )
        xo0s = epip.tile([P, OUTD], f32, tag="xo0s")
        nc.vector.tensor_copy(xo0s[:], xo0p[:, :OUTD])

        # ---- y = elu(dot(xo0, xo_j)) ----
        dscr = scrp.tile([P, OUTD], bf16, tag="dscr")
        d_sb = smallp.tile([P, 1], f32, tag="d")
        nc.vector.scalar_tensor_tensor(
            out=dscr[:], in0=xo[:], scalar=0.0, in1=xo0s[:],
            op0=Alu.bypass, op1=Alu.mult, accum_out=d_sb[:],
        )
        yneg = smallp.tile([P, 1], f32, tag="yneg")
        nc.vector.tensor_scalar_min(yneg[:], d_sb[:], 0.0)
        yexp = smallp.tile([P, 1], f32, tag="yexp")
        nc.scalar.activation(yexp[:], yneg[:], Act.Exp)
        y_sb = smallp.tile([P, 1], f32, tag="y_sb")
        nc.vector.scalar_tensor_tensor(
            out=y_sb[:], in0=yexp[:], scalar=-1.0, in1=d_sb[:],
            op0=Alu.add, op1=Alu.max,
        )
        yrp = tpp.tile([P, P], f32, tag="tp")
        nc.tensor.transpose(yrp[:1, :], y_sb[:], ident[:])
        y_row = epip.tile([1, P], f32, tag="y_row")
        nc.vector.tensor_copy(y_row[:], yrp[:1, :P])
        nc.sync.dma_start(y_out[:], y_row[:])

    nc.compile()
    return nc


def _get_program(T, TB, chunks):
    key = (T, TB, tuple(chunks))
    if key not in _CACHE:
        _CACHE[key] = _build_program(T, TB, chunks)
    return _CACHE[key]


def prepare(x, edge_index, W, att_src, att_dst, bias, item_len):
    """Host-side edge partitioning + feature gather; returns (nc, in_maps, item_len)."""
    import ml_dtypes

    bf16 = ml_dtypes.bfloat16
    item_len = int(np.asarray(item_len))
    x = np.ascontiguousarray(np.asarray(x, np.float32))
    W = np.ascontiguousarray(np.asarray(W, np.float32))
    att_src = np.asarray(att_src, np.float32)
    att_dst = np.asarray(att_dst, np.float32)
    bias = np.asarray(bias, np.float32)
    n_nodes, in_dim = x.shape
    out_dim = W.shape[1]
    assert in_dim == IND and out_dim == OUTD
    assert item_len <= N_CORES * P, "kernel supports item_len <= 1024"

    src = np.asarray(edge_index[0])
    dst = np.asarray(edge_index[1])
    keep = dst < item_len
    src_f = src[keep].astype(np.int32)
    dst_f = dst[keep].astype(np.int32)

    # sort edges by dst; append self-loops so every dst row has >= 1 edge
    loops = np.arange(item_len, dtype=np.int32)
    src_f = np.concatenate([src_f, loops])
    dst_f = np.concatenate([dst_f, loops])
    order = np.argsort(dst_f, kind="stable")
    src_f = src_f[order]
    dst_f = dst_f[order]
    row_start = np.searchsorted(dst_f, np.arange(item_len + 1))

    # per core: groups of 16 same-dst edges, 8 groups per tile
    core_glists = []
    Gmax = 0
    for k in range(N_CORES):
        glist = []  # (dst_local, [srcs])
        for j in range(P):
            row = k * P + j
            if row >= item_len:
                continue
            lo, hi = row_start[row], row_start[row + 1]
            for s in range(lo, hi, 16):
                glist.append((j, src_f[s : min(s + 16, hi)]))
        core_glists.append(glist)
        Gmax = max(Gmax, len(glist))
    T = int(math.ceil(Gmax / 8))
    if T % 2:
        T += 1
    chunks = _chunk_sizes(T)

    # block B: dst == 0 edges incl the (0,0) self-loop, loop moved to slot 0
    b_all = src_f[row_start[0] : row_start[1]]  # graph edges first, loop last
    b_src = np.concatenate([[0], b_all[:-1]]).astype(np.int32)
    nB = len(b_src)
    TB = max(1, int(math.ceil(nB / P)))
    b_pad = TB * P - nB
    b_src = np.concatenate([b_src, np.zeros(b_pad, np.int32)])
    maskB = np.concatenate([np.ones(nB, np.float32), np.zeros(b_pad, np.float32)])
    xbg = x[b_src]
    xb_pack = np.concatenate([xbg, np.ones((TB * P, 1), np.float32)], axis=1)
    xb_pack = (
        xb_pack.reshape(TB, P, IND + 1).transpose(1, 0, 2).reshape(P, TB * (IND + 1))
    )
    xb_bf = np.ascontiguousarray(xb_pack.astype(bf16))
    maskB = np.ascontiguousarray(maskB.reshape(TB, P).T)

    nc = _get_program(T, TB, chunks)

    # chunk-local scatter index offsets
    tile_off = np.zeros(T, np.int32)
    s = 0
    for n in chunks:
        for i in range(n):
            tile_off[s + i] = i * P
        s += n

    av = np.ascontiguousarray(np.stack([att_src, att_dst], axis=1))
    ident = np.eye(P, dtype=np.float32)
    W_bf = np.ascontiguousarray(
        np.concatenate([W[0:P, :], W[P : 2 * P, :]], axis=1).astype(bf16)
    )
    WT = np.ascontiguousarray(W.T)
    bias_r = np.ascontiguousarray(bias.reshape(1, out_dim))

    C = (T + 15) // 16
    in_maps = []
    for k in range(N_CORES):
        glist = core_glists[k]
        # slot arrays
        src_slot = np.zeros((P, T), np.int32)
        dst_slot = np.full((P, T), -1, np.int32)  # -1 = pad
        Dgt = np.zeros((8, T), np.int32)  # gather dst per (group, tile)
        for gi, (j, srcs) in enumerate(glist):
            t = gi // 8
            g = gi % 8
            Dgt[g, t] = j
            rows = 16 * g + np.arange(len(srcs))
            src_slot[rows, t] = srcs
            dst_slot[rows, t] = j
        xg = x[src_slot.T.reshape(-1)]  # [T*P, IND] tile-major
        xg_pack = np.concatenate([xg, np.ones((T * P, 1), np.float32)], axis=1)
        xg_pack = (
            xg_pack.reshape(T, P, IND + 1).transpose(1, 0, 2).reshape(P, T * (IND + 1))
        )
        xg_bf = np.ascontiguousarray(xg_pack.astype(bf16))
        dsti = np.where(dst_slot >= 0, dst_slot + tile_off[None, :], -1).astype(
            np.int16
        )
        dstg = np.zeros((P, C), np.uint16)
        for g in range(8):
            for i in range(T):
                dstg[16 * g + (i % 16), i // 16] = Dgt[g, i]
        mrows = np.minimum(np.arange(k * P, (k + 1) * P, dtype=np.int32), n_nodes - 1)
        x0_bf = np.ascontiguousarray(x[mrows].astype(bf16))
        in_maps.append(
            {
                "xg_in": xg_bf,
                "xb_in": xb_bf,
                "x0_in": x0_bf,
                "dsti_in": np.ascontiguousarray(dsti),
                "dstg_in": np.ascontiguousarray(dstg),
                "maskb_in": maskB,
                "w_in": W_bf,
                "wt_in": WT,
                "av_in": av,
                "bias_in": bias_r,
                "ident_in": ident,
            }
        )
    return nc, in_maps, item_len


def assemble(results, item_len):
    y_all = np.concatenate([results[k]["y_out"].ravel() for k in range(N_CORES)])
    return y_all[1:item_len].astype(np.float32)


def kernel(x, edge_index, W, att_src, att_dst, bias, item_len):
    from concourse import bass_utils

    nc, in_maps, item_len = prepare(
        x, edge_index, W, att_src, att_dst, bias, item_len
    )
    res = bass_utils.run_bass_kernel_spmd(nc, in_maps, core_ids=list(range(N_CORES)))
    return assemble(res.results, item_len)
